# revision 4
# baseline (speedup 1.0000x reference)
"""Multi-head attention (B=4, Q=K=2048, D=512, H=8) on 8 TRN2 NeuronCores.

Sharding: head-parallel with per-batch key truncation.  Core c owns head c
of ALL four batches ("slots").  Each slot's key range is truncated to its
own batch's valid_len (rounded up to 128), so per-core attention work is
sum_b ceil(vl_b/128) chunks instead of 4*max_b chunks -- load-balanced for
any valid_lens distribution, and never worse than batch-parallel.

Per slot the core computes q/k/v projections for its single head, scores,
masked exp (no max pass; exp of -1e6 masked entries underflows to 0),
attn@V with an interleaved ones-column that yields the softmax denominator
for free, normalization via a bf16 C=1 broadcast matmul + fast reciprocal,
and a C=64 output projection against its head's W_o columns, producing a
partial y per batch.  The host sums the 8 cores' partials.  No collectives.

Schedule notes (every engine queue is strict FIFO -- emission order IS the
schedule, and a DMA_DIRECT2D costs ~0.6us of issuing-engine time):
  * all inputs are HOST-PACKED so each tensor is one or two big 2D DMAs
    (~20 DMA instructions total instead of ~100 -- issue serialization on
    the sync engine was costing 60us).
  * slot0's x tensors arrive in two key/q-aligned halves; its first scores
    run ~8us in while the rest streams.
  * slot s+1's q/k projections and each panel's output projection are
    "filler" units popped one per chunk inside the attention streams, so
    the PE absorbs ACT's exp pacing (1147ns/chunk vs PE ~900ns) without
    idling (idle PE re-throttles the HAM clock gate to 1.2 GHz).
  * each panel's normalize is deferred into the next panel's chunk stream;
    v projections run two chunks ahead of their attnV use.
  * y PSUM->SBUF bf16 drains alternate between DVE and ACT (fp32 PSUM
    reads are the scarce shared resource); y DMAs ride the sync queue.
  * 16 dummy matmuls at t=0 warm the PE; a dummy exp preloads ACT tables.
"""

import ml_dtypes
import numpy as np

import concourse.bacc as bacc
import concourse.bass as bass
import concourse.mybir as mybir
from concourse import tile
from concourse.bass_utils import run_bass_kernel_spmd

F32 = mybir.dt.float32
BF16 = mybir.dt.bfloat16

B, Q, KSEQ, D, H = 4, 2048, 2048, 512, 8
DH = D // H          # 64 head dim
NEG = -1.0e6
N_CORES = 8


def build_nc(ktcs):
    """Single-core SPMD program; ktcs = per-slot key-chunk counts (desc)."""
    assert len(ktcs) == B and all(1 <= k <= KSEQ // 128 for k in ktcs)
    NKS = [k * 128 for k in ktcs]
    MOFF = [sum(ktcs[:s]) for s in range(B)]      # mask column offsets
    KH0 = (ktcs[0] + 1) // 2                      # slot0 half sizes (chunks)
    NH0 = KH0 * 128
    L0 = [NH0, NKS[0] - NH0]                      # keys per half
    EXP = mybir.ActivationFunctionType.Exp

    nc = bacc.Bacc("TRN2", target_bir_lowering=False, debug=False,
                   num_devices=N_CORES)

    def din(name, shape, dt=BF16):
        return nc.dram_tensor(name, shape, dt, kind="ExternalInput").ap()

    # host-packed layouts (see make_in_maps): feature-chunk-major columns
    xq_d = [din(f"xq{s}", [128, 4 * Q]) for s in range(B)]
    xk0_d = din("xk0", [128, 4 * NKS[0]])
    xv0_d = din("xv0", [128, 4 * NKS[0]])
    xkv_d = [None] + [din(f"xkv{s}", [128, 8 * NKS[s]]) for s in range(1, B)]
    wqkv_d = din("wqkv", [128, 3 * 4 * DH])
    wo_d = din("wo_t", [DH, D])
    mask_d = din("maskall", [128, sum(ktcs)], F32)
    y_d = [nc.dram_tensor(f"y{s}", [128, 8 * Q // 2], BF16,
                          kind="ExternalOutput").ap() for s in range(B)]

    with tile.TileContext(nc) as tc:
        with (
            nc.allow_low_precision(reason="bf16 matmul operands"),
            tc.tile_pool(name="persist", bufs=1) as pp,
            tc.tile_pool(name="cbuf", bufs=1) as cb,
            tc.tile_pool(name="psA", bufs=2, space=bass.MemorySpace.PSUM) as psA,
            tc.tile_pool(name="psS", bufs=2, space=bass.MemorySpace.PSUM) as psS,
            tc.tile_pool(name="psO", bufs=1, space=bass.MemorySpace.PSUM) as psO,
        ):
            # ---- PE warm-up + ACT exp-table preload (both off critical path)
            warm = pp.tile([128, 512], BF16, tag="warm", name="warm")
            nc.vector.memset(warm[:], 0.0)
            for i in range(9):
                wps = psA.tile([128, 512], F32, tag="proj", name="wps")
                nc.tensor.matmul(wps[:], (warm[:, 0:128]), (warm[:]),
                                 start=True, stop=True)
            pwarm = cb.tile([128, 64], BF16, tag="pwarm", name="pwarm")
            nc.scalar.activation(pwarm[:], warm[:, 0:64], EXP, scale=1.0)

            # ---- constants ----
            onescr = pp.tile([128, DH], F32, tag="onescr", name="onescr")
            nc.vector.memset(onescr[:], 1.0)
            ones_sb = pp.tile([65, DH], BF16, tag="ones", name="ones_sb")
            nc.vector.tensor_copy(ones_sb[64:65, :], onescr[64:65, :])

            # ---- input DMAs (sync queue, ordered by first use) ----
            wqkv = pp.tile([128, 12 * DH], BF16, tag="wqkv", name="wqkv")
            wo = pp.tile([DH, D], BF16, tag="wo", name="wo")
            mask_sb = pp.tile([128, sum(ktcs)], F32, tag="mask", name="mask")
            nc.sync.dma_start(wqkv[:], wqkv_d[:])
            nc.sync.dma_start(mask_sb[:], mask_d[:])
            wq = wqkv[:, 0:4 * DH]
            wk = wqkv[:, 4 * DH:8 * DH]
            wv = wqkv[:, 8 * DH:12 * DH]

            xqt = [pp.tile([128, 4 * Q], BF16, tag=f"xq{s}", name=f"xq{s}")
                   for s in range(B)]
            xk0t = pp.tile([128, 4 * NKS[0]], BF16, tag="xk0", name="xk0")
            xv0t = pp.tile([128, 4 * NKS[0]], BF16, tag="xv0", name="xv0")
            xkvt = [None] + [pp.tile([128, 8 * NKS[s]], BF16, tag=f"xkv{s}",
                                     name=f"xkv{s}") for s in range(1, B)]
            # slot0's first halves ride the sync ring (first scores ~11us in);
            # the bulk rides the ACT-engine ring CONCURRENTLY (issued before
            # any exp, dependency-free) so late slots never starve -- all
            # transfers of one issuing engine serialize on one hardware ring
            nc.sync.dma_start(xqt[0][:, 0:4096], xq_d[0][:, 0:4096])
            nc.sync.dma_start(xk0t[:, 0:4 * L0[0]], xk0_d[:, 0:4 * L0[0]])
            nc.sync.dma_start(xv0t[:, 0:4 * L0[0]], xv0_d[:, 0:4 * L0[0]])
            nc.sync.dma_start(xqt[0][:, 4096:8192], xq_d[0][:, 4096:8192])
            if L0[1]:
                nc.sync.dma_start(xk0t[:, 4 * L0[0]:], xk0_d[:, 4 * L0[0]:])
                nc.sync.dma_start(xv0t[:, 4 * L0[0]:], xv0_d[:, 4 * L0[0]:])
            nc.sync.dma_start(wo[:], wo_d[:])
            for s in range(1, B):
                nc.sync.dma_start(xqt[s][:], xq_d[s][:])
                nc.sync.dma_start(xkvt[s][:], xkv_d[s][:])

            # column address of q position / key position per feature-chunk i
            def xq_ap(s, i, q0, w):
                if s == 0:
                    h = q0 // 1024
                    return xqt[0][:, h * 4096 + i * 1024 + (q0 - h * 1024):
                                  h * 4096 + i * 1024 + (q0 - h * 1024) + w]
                return xqt[s][:, i * Q + q0:i * Q + q0 + w]

            def xk_ap(s, i, c0, w):
                if s == 0:
                    h = 0 if c0 < NH0 else 1
                    base = h * 4 * L0[0] + i * L0[h] + (c0 - h * NH0)
                    return xk0t[:, base:base + w]
                nk = NKS[s]
                return xkvt[s][:, i * 2 * nk + c0:i * 2 * nk + c0 + w]

            def xv_ap(s, i, c0, w):
                if s == 0:
                    h = 0 if c0 < NH0 else 1
                    base = h * 4 * L0[0] + i * L0[h] + (c0 - h * NH0)
                    return xv0t[:, base:base + w]
                nk = NKS[s]
                return xkvt[s][:, i * 2 * nk + nk + c0:i * 2 * nk + nk + c0 + w]

            def kblocks(s):
                """512-wide key blocks, aligned to slot0's half boundary."""
                edges = sorted({0, NKS[s]} | ({NH0} if s == 0 else set()))
                out = []
                for a, b in zip(edges, edges[1:]):
                    for c0 in range(a, b, 512):
                        out.append((c0, min(512, b - c0)))
                return out

            # ---- projections ----
            q_t = [pp.tile([DH, Q], BF16, tag=f"q_t{s}", name=f"q_t{s}")
                   for s in range(B)]
            k_t = [pp.tile([DH, NKS[s]], BF16, tag=f"k_t{s}", name=f"k_t{s}")
                   for s in range(B)]
            v_sb = [pp.tile([128, ktcs[s] * 65], BF16, tag=f"v{s}",
                            name=f"v{s}") for s in range(B)]

            def qproj(s, qs):
                ps = psA.tile([128, 512], F32, tag="proj", name="ps")
                for ic in range(4):
                    nc.tensor.matmul(
                        ps[0:DH, :],
                        (wq[:, ic * DH:(ic + 1) * DH]),
                        (xq_ap(s, ic, qs * 512, 512)),
                        start=(ic == 0), stop=(ic == 3))
                nc.vector.tensor_copy(q_t[s][:, qs * 512:(qs + 1) * 512],
                                      ps[0:DH, :])

            def kproj(s, b0, w):
                ps = psA.tile([128, 512], F32, tag="proj", name="ps")
                for ic in range(4):
                    nc.tensor.matmul(
                        ps[0:DH, :w],
                        (wk[:, ic * DH:(ic + 1) * DH]),
                        (xk_ap(s, ic, b0, w)),
                        start=(ic == 0), stop=(ic == 3))
                nc.vector.tensor_copy(k_t[s][:, b0:b0 + w], ps[0:DH, :w])

            def vproj(s, kt):
                ps = psA.tile([128, 512], F32, tag="proj", name="ps")
                for ic in range(4):
                    nc.tensor.matmul(
                        ps[:, 0:DH],
                        (xv_ap(s, ic, kt * 128, 128)),
                        (wv[:, ic * DH:(ic + 1) * DH]),
                        start=(ic == 0), stop=(ic == 3))
                nc.vector.tensor_copy(v_sb[s][:, kt * 65:kt * 65 + 64],
                                      ps[:, 0:DH])

            def proj_doses(s, qs_from=0, kb_from=0):
                ds = [(lambda qs=qs, s=s: qproj(s, qs))
                      for qs in range(qs_from, 4)]
                ds += [(lambda b0=b0, w=w, s=s: kproj(s, b0, w))
                       for (b0, w) in kblocks(s)[kb_from:]]
                return ds

            # upfront: only what the first scores chunk needs
            qproj(0, 0)
            qproj(0, 1)
            nb0 = len([b for (b, w) in kblocks(0) if b < NH0])
            for (b0, w) in kblocks(0)[:nb0]:
                kproj(0, b0, w)

            # ---- attention ----
            o_sb = [pp.tile([DH, Q], BF16, tag=f"o{s}", name=f"o{s}")
                    for s in range(B)]
            pending = [None]   # normalize closure for the previous panel
            dose_fifo = []     # projections with a hard deadline
            out_fifo = []      # output-projection units (soft deadline)
            ucount = [0]

            def outproj_unit(s, q0, ot, yst):
                def unit():
                    for qh in range(2):
                        y_ps = psA.tile([128, 512], F32, tag="proj",
                                        name="y_ps")
                        nc.tensor.matmul(
                            y_ps[:],
                            (wo[:, ot * 128:(ot + 1) * 128]),
                            (o_sb[s][:, q0 + qh * 512:q0 + (qh + 1) * 512]),
                            start=True, stop=True)
                        ucount[0] += 1
                        dst = yst[:, ot * 1024 + qh * 512:
                                  ot * 1024 + (qh + 1) * 512]
                        if ucount[0] % 2 == 0:
                            nc.scalar.copy(dst, y_ps[:])
                        else:
                            nc.vector.tensor_copy(dst, y_ps[:])
                    if ot == 3:
                        panel = q0 // 1024
                        nc.sync.dma_start(
                            y_d[s][:, panel * 4096:(panel + 1) * 4096],
                            yst[:])
                return unit

            def make_finish(s, q0, oA, oB):
                def fin():
                    # normalize: o[dh, q] /= denom[q] (row 64 of oA/oB);
                    # bf16 denominator row -> bf16 broadcast matmul
                    for hf, o_ps in enumerate((oA, oB)):
                        dn = cb.tile([65, 512], BF16, tag="dn", bufs=2,
                                     name="dn")
                        nc.vector.tensor_copy(dn[64:65, :], o_ps[64:65, :])
                        bc_ps = psA.tile([64, 512], F32, tag="proj",
                                         name="bc_ps")
                        nc.tensor.matmul(bc_ps[:], (ones_sb[64:65, :]),
                                         (dn[64:65, :]), start=True, stop=True)
                        inv_sb = cb.tile([64, 512], F32, tag="invb", bufs=2,
                                         name="inv_sb")
                        nc.vector.reciprocal_approx_fast(inv_sb[:], bc_ps[:])
                        cols = slice(q0 + hf * 512, q0 + (hf + 1) * 512)
                        nc.vector.tensor_mul(o_sb[s][:, cols],
                                             o_ps[0:DH, :], inv_sb[:])
                    yst = cb.tile([128, 4096], BF16, tag="yst", bufs=2,
                                  name="yst")
                    out_fifo.extend(outproj_unit(s, q0, ot, yst)
                                    for ot in range(4))
                return fin

            for s in range(B):
                KTC = ktcs[s]
                # this slot's projections MUST be emitted before its scores
                for f in dose_fifo:
                    f()
                dose_fifo.clear()
                if s == 0:
                    # slot0's own second-half projections dose into panel0
                    dose_fifo.extend(proj_doses(0, qs_from=2, kb_from=nb0))
                # v ones-columns, once per slot (gives softmax denominator)
                nc.vector.tensor_copy(v_sb[s][:, 64::65], onescr[:, 0:KTC])
                for panel in range(2):
                    if panel == 1:
                        for f in dose_fifo:   # slot0 leftovers: p1 needs them
                            f()
                        dose_fifo.clear()
                        if s + 1 < B:
                            dose_fifo.extend(proj_doses(s + 1))
                    q0 = panel * 1024
                    oA = psO.tile([65, 512], F32, tag="oA", name="oA")
                    oB = psO.tile([65, 512], F32, tag="oB", name="oB")

                    def attnv(p, kt, s=s, oA=oA, oB=oB, KTC=KTC):
                        for hf, o_ps in enumerate((oA, oB)):
                            nc.tensor.matmul(
                                o_ps[:],
                                (v_sb[s][:, kt * 65:kt * 65 + 65]),
                                (p[:, hf * 512:(hf + 1) * 512]),
                                start=(kt == 0), stop=(kt == KTC - 1))

                    # v projections run 2 chunks ahead of their attnv use
                    if panel == 0:
                        vproj(s, 0)
                        if KTC > 1:
                            vproj(s, 1)
                    prev = None
                    for kt in range(KTC):
                        if panel == 0 and kt + 2 < KTC:
                            vproj(s, kt + 2)
                        s_ps = psS.tile([128, 1024], F32, tag="s", name="s_ps")
                        for hf in range(2):
                            nc.tensor.matmul(
                                s_ps[:, hf * 512:(hf + 1) * 512],
                                (k_t[s][:, kt * 128:(kt + 1) * 128]),
                                (q_t[s][:, q0 + hf * 512:q0 + (hf + 1) * 512]),
                                start=True, stop=True)
                        p_sb = cb.tile([128, 1024], BF16, tag="p", bufs=6,
                                       name="p_sb")
                        nc.scalar.activation(
                            p_sb[:], s_ps[:], EXP,
                            bias=mask_sb[:, MOFF[s] + kt:MOFF[s] + kt + 1],
                            scale=1.0)
                        if kt == 0 and pending[0] is not None:
                            pending[0]()
                            pending[0] = None
                        if prev is not None:
                            attnv(*prev)
                        if dose_fifo and (panel == 1 or kt >= 4):
                            dose_fifo.pop(0)()
                        elif out_fifo and (panel == 1 or len(out_fifo) > 2):
                            out_fifo.pop(0)()
                            # drain deep backlog 2-per-chunk so it doesn't
                            # spill into a serial flush at the very end
                            if len(out_fifo) > 6:
                                out_fifo.pop(0)()
                        prev = (p_sb, kt)
                    attnv(*prev)
                    pending[0] = make_finish(s, q0, oA, oB)
            pending[0]()
            for f in out_fifo:
                f()

    nc.compile()
    return nc


def plan(valid_lens):
    """Slot order (batches sorted by descending chunk count) + chunk counts."""
    vl = np.asarray(valid_lens).astype(np.int64)
    ktc = [max(1, int((int(v) + 127) // 128)) for v in vl]
    order = sorted(range(B), key=lambda b: -ktc[b])
    return order, tuple(ktc[b] for b in order)


def make_in_maps(queries, keys, values, valid_lens, W_q, W_k, W_v, W_o,
                 order, ktcs):
    bf = ml_dtypes.bfloat16
    queries = np.asarray(queries, np.float32)
    keys = np.asarray(keys, np.float32)
    values = np.asarray(values, np.float32)
    W_q = np.asarray(W_q, np.float32)
    W_k = np.asarray(W_k, np.float32)
    W_v = np.asarray(W_v, np.float32)
    W_o = np.asarray(W_o, np.float32)
    vl = np.asarray(valid_lens).astype(np.int64)
    KH0 = (ktcs[0] + 1) // 2
    NH0 = KH0 * 128

    def packw(w):  # [64, 512] head slice -> lhsT chunks packed [128, 256]
        wt = np.ascontiguousarray(w.T)          # [512, 64]
        return np.concatenate([wt[i * 128:(i + 1) * 128, :] for i in range(4)],
                              axis=1).astype(bf)

    def chunkcat(a, col_ranges):  # a: [512, N] -> [128, 4*sum(w)] packed
        parts = []
        for (c0, c1) in col_ranges:
            parts.append(np.concatenate(
                [a[i * 128:(i + 1) * 128, c0:c1] for i in range(4)], axis=1))
        return np.ascontiguousarray(np.concatenate(parts, axis=1))

    common = {}
    masks = []
    for s, b in enumerate(order):
        nk = ktcs[s] * 128
        qT = queries[b].T.astype(bf)
        kT = keys[b, :nk].T.astype(bf)
        vT = values[b, :nk].T.astype(bf)
        if s == 0:
            common["xq0"] = chunkcat(qT, [(0, 1024), (1024, 2048)])
            common["xk0"] = chunkcat(kT, [(0, NH0), (NH0, nk)] if nk > NH0
                                     else [(0, NH0)])
            common["xv0"] = chunkcat(vT, [(0, NH0), (NH0, nk)] if nk > NH0
                                     else [(0, NH0)])
        else:
            common[f"xq{s}"] = chunkcat(qT, [(0, 2048)])
            kv = np.concatenate([
                np.concatenate([kT[i * 128:(i + 1) * 128, :],
                                vT[i * 128:(i + 1) * 128, :]], axis=1)
                for i in range(4)], axis=1)
            common[f"xkv{s}"] = np.ascontiguousarray(kv)
        m = np.where(np.arange(nk) < vl[b], 0.0, NEG).astype(np.float32)
        masks.append(m.reshape(ktcs[s], 128).T)
    common["maskall"] = np.ascontiguousarray(np.concatenate(masks, axis=1))

    in_maps = []
    for c in range(N_CORES):
        sl = slice(c * DH, (c + 1) * DH)
        im = dict(common)
        im["wqkv"] = np.ascontiguousarray(np.concatenate(
            [packw(W_q[sl, :] / 8.0), packw(W_k[sl, :]), packw(W_v[sl, :])],
            axis=1))
        im["wo_t"] = np.ascontiguousarray(W_o[:, sl].T).astype(bf)
        in_maps.append(im)
    return in_maps


def unpack_y(arr):
    """[128, 8192] device layout -> [512, 2048] partial y."""
    y = np.empty((D, Q), np.float32)
    a = np.asarray(arr, dtype=np.float32)
    for panel in range(2):
        for ot in range(4):
            y[ot * 128:(ot + 1) * 128, panel * 1024:(panel + 1) * 1024] = \
                a[:, panel * 4096 + ot * 1024:panel * 4096 + (ot + 1) * 1024]
    return y


def kernel(queries, keys, values, valid_lens, W_q, W_k, W_v, W_o):
    order, ktcs = plan(valid_lens)
    nc = build_nc(ktcs)
    in_maps = make_in_maps(queries, keys, values, valid_lens,
                           W_q, W_k, W_v, W_o, order, ktcs)
    res = run_bass_kernel_spmd(nc, in_maps, list(range(N_CORES))).results
    out = np.zeros((B, Q, D), np.float32)
    for s, b in enumerate(order):
        acc = np.zeros((D, Q), np.float32)
        for c in range(N_CORES):
            acc += unpack_y(res[c][f"y{s}"])
        out[b] = acc.T
    return out
